# revision 21
# baseline (speedup 1.0000x reference)
"""Trainium2 Bass kernel for a LISTA layer (nn_ListaLayer).

Reference computation (jax, fp32):
    th = relu(Theta) + 1e-7
    xW = (y @ W) / th
    repeat 16: z = xW + (unit_threshold(z) * th @ S) / th
    out = (unit_threshold(z) * th) @ Dx
where unit_threshold(v) = sign(v) * relu(|v| - 1).

Algebraic restructure (exact): track v = z * th.  Then
    v0 = y @ W
    repeat 16:  u = soft_threshold(v, th) = sign(v) * relu(|v| - th)
                v = v0 + u @ S
    out = soft_threshold(v, th) @ Dx

Precision schedule (validated against an fp64 reference emulation,
rel-err ~8.5e-3 vs the 2e-2 gate):
  - A (v0 = y@W): single-pass fp16.
  - steps 1..13: S and u in fp8-e4m3 (S scaled by 512), single pass,
    using the DoubleRow perf mode (contracts 2x128 per instr at 0.5
    cycles/row -> 4x the fp16 matmul rate).
  - steps 14..16: 3-pass split-fp8 (uh@Sh + uh@Sl + ul@Sh), DoubleRow.
  - C (a @ Dx): single-pass fp16.
The v = v0 + u@S add rides the PE accumulation chain: an extra
DoubleRow matmul with stationary (128*I, 8*I) and moving (v0h8, v0l8),
where v0h8 = fp8(4*v0), v0l8 = fp8(4*v0 - v0h8); contribution =
512*v0 to within ~2^-9, matching the psum scale of 512*(u@S).

shrink on-chip: psum holds 512*v.
  ACT path: p = relu(psum/512 - th), q = relu(-psum/512 - th) [fp16],
            u8 = p - q (DVE, fp8 out).
  DVE path (4 of 16 tiles, for ACT/DVE balance): u = max(v,th)+min(v,-th):
            t1 = (psum max 512th) * 2^-9, t2 = (psum min -512th) * 2^-9,
            u8 = t1 + t2.

Distribution: data-parallel over batch rows, 8 cores, 2048 rows each;
weights replicated; no collectives.  Per-core loop is split into two
batch halves of 1024 columns so elementwise of one half overlaps the
other half's matmuls; u tiles double-buffer via tile-pool tag rings.
"""

import numpy as np
import ml_dtypes
from contextlib import ExitStack

import concourse.bass as bass
import concourse.bacc as bacc
import concourse.tile as tile
import concourse.mybir as mybir
from concourse.bass import ts, ds

P = 128
NCORES = 8
B_FULL, DIN, DD = 16384, 1024, 2048
BSH = B_FULL // NCORES      # 2048 batch rows per core
HB = BSH // 2               # 1024: batch half (elementwise granularity)
MC = 256                    # matmul moving chunk (DR rhs free = 2*256 = 512)
IT = DD // P                # 16 dict tiles
KW = DIN // P               # 8 d_in tiles
ACH = 1024                  # phase-A batch chunk (= one half)
SA = 512.0                  # S scale (fp8 subnormal escape)
DVE_I = (3, 7, 11, 15)      # prefix tiles shrunk on the DVE path

F8np = ml_dtypes.float8_e4m3
F8 = mybir.dt.float8e4
F16 = mybir.dt.float16
F32 = mybir.dt.float32
ADD = mybir.AluOpType.add
SUB = mybir.AluOpType.subtract
MAX = mybir.AluOpType.max
MIN = mybir.AluOpType.min
MULT = mybir.AluOpType.mult
RELU = mybir.ActivationFunctionType.Relu
COPY = mybir.ActivationFunctionType.Copy
DR = mybir.MatmulPerfMode.DoubleRow

_built = {}


def _build(steps: int):
    """Trace + schedule + compile the SPMD kernel for `steps` unroll steps."""
    assert steps >= 4, "schedule assumes >= 4 steps"
    n1 = steps - 3          # one-pass fp8 steps (incl. step 1 done in A)

    nc = bacc.Bacc("TRN2", target_bir_lowering=False, debug=False, num_devices=NCORES)

    def inp(name, shape, dt):
        return nc.dram_tensor(name, shape, dt, kind="ExternalInput").ap()

    yT16_d = inp("yT16", (DIN, BSH), F16)
    W16_d = inp("W16", (DIN, DD), F16)
    S8h_d = inp("S8h", (DD, DD), F8)
    S8l_d = inp("S8l", (DD, DD), F8)
    Dx16_d = inp("Dx16", (DD, DIN), F16)
    eye_d = inp("eye2", (P, 2, P), F8)       # (128*I, 8*I)
    nth_d = inp("nth", (DD,), F32)           # -th
    th512_d = inp("th512", (DD,), F32)       # +512*th
    nth512_d = inp("nth512", (DD,), F32)     # -512*th
    out_d = nc.dram_tensor("out", (BSH, DIN), F32, kind="ExternalOutput").ap()

    with tile.TileContext(nc) as tc, ExitStack() as top:
        dram = top.enter_context(tc.tile_pool(name="dram", bufs=1, space="DRAM"))
        v08_spill = dram.tile([IT, P, 2, BSH], F8)
        a16_spill = dram.tile([IT, P, BSH], F16)

        thp = top.enter_context(tc.tile_pool(name="thp", bufs=1))
        nth_t = thp.tile([P, IT], F32)
        th512_t = thp.tile([P, IT], F32)
        nth512_t = thp.tile([P, IT], F32)
        eye_t = thp.tile([P, 2, P], F8)
        nc.sync.dma_start(nth_t[:], nth_d.rearrange("(io p) -> p io", p=P))
        nc.sync.dma_start(th512_t[:], th512_d.rearrange("(io p) -> p io", p=P))
        nc.sync.dma_start(nth512_t[:], nth512_d.rearrange("(io p) -> p io", p=P))
        nc.sync.dma_start(eye_t[:], eye_d)

        with ExitStack() as bigctx:
            spool = bigctx.enter_context(tc.tile_pool(name="spool", bufs=1))
            S8h_t = spool.tile([P, IT, DD], F8, name="S8h_t")

            upool = bigctx.enter_context(tc.tile_pool(name="upool", bufs=2))
            psAB = bigctx.enter_context(tc.tile_pool(name="psAB", bufs=4, space="PSUM"))
            u8_h = [upool.tile([P, IT, HB], F8, tag=f"u8h{h}", name=f"u8_h{h}")
                    for h in range(2)]

            v08p = bigctx.enter_context(tc.tile_pool(name="v08p", bufs=3))
            pq = bigctx.enter_context(tc.tile_pool(name="pq", bufs=3))
            uh8_h = [None, None]
            ul8_h = [None, None]
            cur = {}
            # tailp/s8lp opened lazily (first 3-pass step) to keep phase-A SBUF low
            lazy = {}

            def step_begin(k, h):
                one_pass = k <= n1
                split_prod = (k + 1 > n1) and (k < steps)
                final = k == steps
                cur[h] = (u8_h[h], uh8_h[h], ul8_h[h])
                if split_prod:
                    uh8_h[h] = upool.tile([P, IT, HB], F8, tag=f"u8h{h}", name="uh8_n")
                    ul8_h[h] = lazy["tailp"].tile([P, IT, HB], F8, tag=f"ul8h{h}",
                                                  name="ul8_n")
                elif not final:
                    u8_h[h] = upool.tile([P, IT, HB], F8, tag=f"u8h{h}", name="u8_n")

            def step_tile(k, h, i):
                one_pass = k <= n1
                split_prod = (k + 1 > n1) and (k < steps)
                final = k == steps
                hs = ds(h * HB, HB)
                u8_cur, uh8_cur, ul8_cur = cur[h]
                v08c = v08p.tile([P, 2, HB], F8, tag="v08", name="v08c")
                dq = nc.sync if i % 2 else nc.scalar
                dq.dma_start(v08c[:], v08_spill[i, :, :, hs])
                if not one_pass:
                    s8li = lazy["s8lp"].tile([P, IT, P], F8, tag="s8l", name="s8li")
                    dq2 = nc.scalar if i % 2 else nc.sync
                    dq2.dma_start(s8li[:], S8l_d[:, ts(i, P)].rearrange(
                        "(jo p) c -> p jo c", p=P))
                ps = psAB.tile([P, HB], F32, tag="ps", name="psB_t")
                for c in range(HB // MC):
                    cl = ds(c * MC, MC)
                    nc.tensor.matmul(ps[:, cl], eye_t[:], v08c[:, :, cl],
                                     start=True, stop=False, perf_mode=DR)
                    if one_pass:
                        for jp in range(IT // 2):
                            nc.tensor.matmul(
                                ps[:, cl], S8h_t[:, 2 * jp:2 * jp + 2, ts(i, P)],
                                u8_cur[:, 2 * jp:2 * jp + 2, cl],
                                start=False, stop=(jp == IT // 2 - 1), perf_mode=DR)
                    else:
                        for jp in range(IT // 2):
                            nc.tensor.matmul(
                                ps[:, cl], S8h_t[:, 2 * jp:2 * jp + 2, ts(i, P)],
                                uh8_cur[:, 2 * jp:2 * jp + 2, cl],
                                start=False, stop=False, perf_mode=DR)
                        for jp in range(IT // 2):
                            nc.tensor.matmul(
                                ps[:, cl], s8li[:, 2 * jp:2 * jp + 2, :],
                                uh8_cur[:, 2 * jp:2 * jp + 2, cl],
                                start=False, stop=False, perf_mode=DR)
                        for jp in range(IT // 2):
                            nc.tensor.matmul(
                                ps[:, cl], S8h_t[:, 2 * jp:2 * jp + 2, ts(i, P)],
                                ul8_cur[:, 2 * jp:2 * jp + 2, cl],
                                start=False, stop=(jp == IT // 2 - 1), perf_mode=DR)

                # shrink: psum holds 512*v
                if (not split_prod) and (not final) and i in DVE_I:
                    # DVE path: u = max(v,th) + min(v,-th)
                    t1 = pq.tile([P, HB], F16, tag="p", name="t1")
                    t2 = pq.tile([P, HB], F16, tag="q", name="t2")
                    nc.vector.tensor_scalar(
                        t1[:], ps[:], th512_t[:, i:i + 1], 1.0 / SA, MAX, MULT)
                    nc.vector.tensor_scalar(
                        t2[:], ps[:], nth512_t[:, i:i + 1], 1.0 / SA, MIN, MULT)
                    nc.vector.tensor_tensor(u8_h[h][:, i, :], t1[:], t2[:], ADD)
                else:
                    p_t = pq.tile([P, HB], F16, tag="p", name="p_t")
                    q_t = pq.tile([P, HB], F16, tag="q", name="q_t")
                    nc.scalar.activation(p_t[:], ps[:], RELU,
                                         bias=nth_t[:, i:i + 1], scale=1.0 / SA)
                    nc.scalar.activation(q_t[:], ps[:], RELU,
                                         bias=nth_t[:, i:i + 1], scale=-1.0 / SA)
                    if final:
                        u16 = pq.tile([P, HB], F16, tag="u16", name="u16")
                        nc.vector.tensor_tensor(u16[:], p_t[:], q_t[:], SUB)
                        nc.sync.dma_start(a16_spill[i, :, hs], u16[:])
                    elif split_prod:
                        u16 = pq.tile([P, HB], F16, tag="u16", name="u16")
                        nc.vector.tensor_tensor(u16[:], p_t[:], q_t[:], SUB)
                        nc.vector.tensor_copy(uh8_h[h][:, i, :], u16[:])
                        nc.vector.tensor_tensor(ul8_h[h][:, i, :], u16[:],
                                                uh8_h[h][:, i, :], SUB)
                    else:
                        nc.vector.tensor_tensor(u8_h[h][:, i, :], p_t[:], q_t[:], SUB)

            # ---------------- Phase A: v0 = W^T @ y^T; v0 fp8 pair + u_1 ----------------
            with ExitStack() as actx:
                wpool = actx.enter_context(tc.tile_pool(name="wpool", bufs=1))
                ypool = actx.enter_context(tc.tile_pool(name="ypool", bufs=2))
                vtmp = actx.enter_context(tc.tile_pool(name="vtmp", bufs=3))
                pqA = actx.enter_context(tc.tile_pool(name="pqA", bufs=3))

                W16_t = wpool.tile([P, KW, DD], F16, name="W16_t")
                for ko in range(KW):
                    dq = nc.sync if ko % 2 else nc.scalar
                    dq.dma_start(W16_t[:, ko, :], W16_d[ts(ko, P), :])

                for h in range(BSH // ACH):
                    cs = ds(h * ACH, ACH)
                    yc = ypool.tile([P, KW, ACH], F16, tag="yc", name="yc")
                    for ko in range(KW):
                        dq = nc.sync if ko % 2 else nc.scalar
                        dq.dma_start(yc[:, ko, :], yT16_d[ts(ko, P), cs])
                    if h == 0:
                        # S8h needed from step 2 on; queue it behind the first
                        # y chunk so A's first matmuls aren't starved, but well
                        # before the B phase starts
                        for jo in range(IT):
                            dq = nc.sync if jo % 2 else nc.scalar
                            dq.dma_start(S8h_t[:, jo, :], S8h_d[ts(jo, P), :])
                    else:
                        # overlap step-2/h0 with the second A half
                        step_begin(2, 0)
                    for i in range(IT):
                        ps = psAB.tile([P, ACH], F32, tag="ps", name="psA_t")
                        for s in range(2):
                            sl = ds(s * 512, 512)
                            for ko in range(KW):
                                nc.tensor.matmul(ps[:, sl], W16_t[:, ko, ts(i, P)],
                                                 yc[:, ko, sl],
                                                 start=(ko == 0), stop=(ko == KW - 1))
                        # v0 pair: v0h8 = fp8(4*v0), v0l8 = fp8(4*v0 - v0h8)
                        # (eye carries 128x on both -> contribution 512*v0)
                        v0h8c = vtmp.tile([P, ACH], F8, tag="v0h8", name="v0h8c")
                        if i % 2:
                            nc.scalar.activation(v0h8c[:], ps[:], COPY, scale=4.0)
                        else:
                            nc.vector.tensor_scalar(v0h8c[:], ps[:], 4.0, None, MULT)
                        v0l8c = vtmp.tile([P, ACH], F8, tag="v0l8", name="v0l8c")
                        nc.vector.scalar_tensor_tensor(v0l8c[:], ps[:], 4.0, v0h8c[:],
                                                       MULT, SUB)
                        nc.sync.dma_start(v08_spill[i, :, 0, cs], v0h8c[:])
                        nc.scalar.dma_start(v08_spill[i, :, 1, cs], v0l8c[:])
                        # u_1 = shrink(v0):  psum is UNSCALED here (scale=1)
                        p_t = pqA.tile([P, ACH], F16, tag="pA", name="p_t")
                        q_t = pqA.tile([P, ACH], F16, tag="qA", name="q_t")
                        nc.scalar.activation(p_t[:], ps[:], RELU, bias=nth_t[:, i:i + 1],
                                             scale=1.0)
                        nc.scalar.activation(q_t[:], ps[:], RELU, bias=nth_t[:, i:i + 1],
                                             scale=-1.0)
                        nc.vector.tensor_tensor(u8_h[h][:, i, :], p_t[:], q_t[:], SUB)
                        if h == 1:
                            step_tile(2, 0, i)

            # ---------------- Phase B ----------------
            with ExitStack() as bctx:
                lazy["tailp"] = bctx.enter_context(tc.tile_pool(name="tailp", bufs=2))
                lazy["s8lp"] = bctx.enter_context(tc.tile_pool(name="s8lp", bufs=3))

                # step k consumes u_k, produces u_{k+1} (or a16 at k=steps)
                for k in range(2, steps + 1):
                    for h in range(2):
                        if k == 2 and h == 0:
                            continue    # emitted inside phase A
                        step_begin(k, h)
                        for i in range(IT):
                            step_tile(k, h, i)

        # ---------------- Phase C: out = a @ Dx (fp16) ----------------
        with ExitStack() as cctx:
            cpool = cctx.enter_context(tc.tile_pool(name="cpool", bufs=1))
            psC = cctx.enter_context(tc.tile_pool(name="psC", bufs=4, space="PSUM"))
            stC = cctx.enter_context(tc.tile_pool(name="stC", bufs=3))

            CN = 512
            Dx16_t = cpool.tile([P, IT, DIN], F16, name="Dx16_t")
            for dn in range(DIN // CN):
                for io in range(IT):
                    nc.scalar.dma_start(Dx16_t[:, io, ts(dn, CN)],
                                        Dx16_d[ts(io, P), ts(dn, CN)])
            a16p2 = cctx.enter_context(tc.tile_pool(name="a16p2", bufs=3))
            for bt in range(BSH // P):
                a16bt = a16p2.tile([P, IT, P], F16, tag="a16bt", name="a16bt")
                nc.sync.dma_start(a16bt[:], a16_spill[:, :, ts(bt, P)].rearrange(
                    "io p b -> p io b"))
                for dn in range(DIN // CN):
                    ps = psC.tile([P, CN], F32, tag="psC", name="psC_t")
                    for io in range(IT):
                        nc.tensor.matmul(ps[:], a16bt[:, io, :],
                                         Dx16_t[:, io, ts(dn, CN)],
                                         start=(io == 0), stop=(io == IT - 1))
                    st = stC.tile([P, CN], F32, tag="stC", name="st")
                    nc.scalar.activation(st[:], ps[:], COPY)
                    nc.sync.dma_start(out_d[ts(bt, P), ts(dn, CN)], st[:])

    nc.compile()
    return nc


def _prep_in_maps(y, W, Theta, S, Dx):
    y = np.ascontiguousarray(np.asarray(y, dtype=np.float32))
    W = np.asarray(W, dtype=np.float32)
    Theta = np.asarray(Theta, dtype=np.float32)
    S = np.asarray(S, dtype=np.float32)
    Dx = np.asarray(Dx, dtype=np.float32)
    assert y.shape == (B_FULL, DIN) and W.shape == (DIN, DD)
    assert S.shape == (DD, DD) and Dx.shape == (DD, DIN)

    W16 = W.astype(np.float16)
    S512 = S * np.float32(SA)
    S8h = S512.astype(F8np)
    S8l = (S512 - S8h.astype(np.float32)).astype(F8np)
    Dx16 = Dx.astype(np.float16)
    th = (np.maximum(Theta, 0.0) + np.float32(1e-7)).astype(np.float32)
    eye = np.zeros((P, 2, P), np.float32)
    eye[:, 0, :] = np.eye(P) * 128.0
    eye[:, 1, :] = np.eye(P) * 128.0
    eye2 = eye.astype(F8np)
    yT16 = np.ascontiguousarray(y.T).astype(np.float16)

    shared = dict(W16=W16, S8h=S8h, S8l=S8l, Dx16=Dx16, eye2=eye2,
                  nth=-th, th512=np.float32(SA) * th, nth512=-np.float32(SA) * th)
    in_maps = []
    for c in range(NCORES):
        sl = slice(c * BSH, (c + 1) * BSH)
        in_maps.append(dict(shared, yT16=np.ascontiguousarray(yT16[:, sl])))
    return in_maps


_sharded_cache = {}


def _get_sharded(steps: int):
    """Build (once) the jitted shard_map executable for the compiled NEFF."""
    if steps in _sharded_cache:
        return _sharded_cache[steps]
    import jax
    from jax.experimental.shard_map import shard_map
    from jax.sharding import Mesh, PartitionSpec
    from concourse import bass2jax

    if steps not in _built:
        _built[steps] = _build(steps)
    nc = _built[steps]
    bass2jax.install_neuronx_cc_hook()
    assert nc.dbg_addr is None
    partition_name = nc.partition_id_tensor.name if nc.partition_id_tensor else None

    in_names, out_names, out_avals, zero_shapes = [], [], [], []
    for alloc in nc.m.functions[0].allocations:
        if not isinstance(alloc, mybir.MemoryLocationSet):
            continue
        name = alloc.memorylocations[0].name
        if alloc.kind == "ExternalInput":
            if name != partition_name:
                in_names.append(name)
        elif alloc.kind == "ExternalOutput":
            out_names.append(name)
            shape = tuple(alloc.tensor_shape)
            dtype = mybir.dt.np(alloc.dtype)
            out_avals.append(jax.core.ShapedArray(shape, dtype))
            zero_shapes.append((shape, dtype))
    n_params = len(in_names)
    n_outs = len(out_names)
    all_in_names = in_names + out_names
    if partition_name is not None:
        all_in_names.append(partition_name)

    def _body(*args):
        operands = list(args)
        if partition_name is not None:
            operands.append(bass2jax.partition_id_tensor())
        outs = bass2jax._bass_exec_p.bind(
            *operands,
            out_avals=tuple(out_avals),
            in_names=tuple(all_in_names),
            out_names=tuple(out_names),
            lowering_input_output_aliases=(),
            sim_require_finite=True,
            sim_require_nnan=True,
            nc=nc,
        )
        return tuple(outs)

    devices = jax.devices()[:NCORES]
    mesh = Mesh(np.asarray(devices), ("core",))
    donate = tuple(range(n_params, n_params + n_outs))
    sharded = jax.jit(
        shard_map(_body, mesh=mesh,
                  in_specs=(PartitionSpec("core"),) * (n_params + n_outs),
                  out_specs=(PartitionSpec("core"),) * n_outs,
                  check_rep=False),
        donate_argnums=donate, keep_unused=True)
    entry = dict(sharded=sharded, in_names=in_names, out_names=out_names,
                 zero_shapes=zero_shapes, mesh=mesh, n_params=n_params)
    _sharded_cache[steps] = entry
    return entry


def _concat_inputs(entry, in_maps):
    return [np.concatenate([np.asarray(in_maps[c][n]) for c in range(NCORES)], axis=0)
            for n in entry["in_names"]]


def _run(entry, concat_in):
    zeros = [np.zeros((NCORES * s[0], *s[1:]), d) for s, d in entry["zero_shapes"]]
    out_arrs = entry["sharded"](*concat_in, *zeros)
    return out_arrs


def kernel(y, W, Theta, S, Dx, unroll_steps):
    steps = int(unroll_steps)
    entry = _get_sharded(steps)
    in_maps = _prep_in_maps(y, W, Theta, S, Dx)
    out_arrs = _run(entry, _concat_inputs(entry, in_maps))
    idx = entry["out_names"].index("out")
    return np.ascontiguousarray(np.asarray(out_arrs[idx]))  # [NCORES*BSH, DIN]


def time_kernel(np_inputs, iters=6):
    """Steady-state wall time per NEFF execution (ns), device-resident inputs."""
    import jax
    from jax.sharding import NamedSharding, PartitionSpec
    steps = int(np_inputs["unroll_steps"])
    entry = _get_sharded(steps)
    in_maps = _prep_in_maps(np_inputs["y"], np_inputs["W"], np_inputs["Theta"],
                            np_inputs["S"], np_inputs["Dx"])
    concat_in = _concat_inputs(entry, in_maps)
    sh = NamedSharding(entry["mesh"], PartitionSpec("core"))
    dev_in = [jax.device_put(a, sh) for a in concat_in]
    import time as _time
    times = []
    for it in range(iters):
        zeros = [jax.device_put(np.zeros((NCORES * s[0], *s[1:]), d), sh)
                 for s, d in entry["zero_shapes"]]
        for z in zeros:
            z.block_until_ready()
        t0 = _time.perf_counter()
        outs = entry["sharded"](*dev_in, *zeros)
        for o in outs:
            o.block_until_ready()
        times.append(_time.perf_counter() - t0)
    best = min(times[1:]) if len(times) > 1 else times[0]
    print("  per-iter times (ms):", [f"{t*1e3:.1f}" for t in times])
    return best * 1e9


if __name__ == "__main__":
    rng = np.random.default_rng(0)
    inputs = dict(
        y=rng.standard_normal((B_FULL, DIN), dtype=np.float32),
        W=(rng.standard_normal((DIN, DD)) * 0.02).astype(np.float32),
        Theta=rng.random(DD, dtype=np.float32),
        S=(rng.standard_normal((DD, DD)) * 0.02).astype(np.float32),
        Dx=(rng.standard_normal((DD, DIN)) * 0.02).astype(np.float32),
        unroll_steps=16,
    )
    out = kernel(**inputs)
    print("out", out.shape, out.dtype, np.abs(out).max())


# revision 23
# speedup vs baseline: 1.0101x; 1.0101x over previous
"""Trainium2 Bass kernel for a LISTA layer (nn_ListaLayer).

Reference computation (jax, fp32):
    th = relu(Theta) + 1e-7
    xW = (y @ W) / th
    repeat 16: z = xW + (unit_threshold(z) * th @ S) / th
    out = (unit_threshold(z) * th) @ Dx
where unit_threshold(v) = sign(v) * relu(|v| - 1).

Algebraic restructure (exact): track v = z * th.  Then
    v0 = y @ W
    repeat 16:  u = soft_threshold(v, th) = sign(v) * relu(|v| - th)
                v = v0 + u @ S
    out = soft_threshold(v, th) @ Dx

Precision schedule (validated against an fp64 reference emulation,
rel-err ~8.5e-3 vs the 2e-2 gate):
  - A (v0 = y@W): single-pass fp16.
  - steps 1..13: S and u in fp8-e4m3 (S scaled by 512), single pass,
    using the DoubleRow perf mode (contracts 2x128 per instr at 0.5
    cycles/row -> 4x the fp16 matmul rate).
  - steps 14..16: 3-pass split-fp8 (uh@Sh + uh@Sl + ul@Sh), DoubleRow.
  - C (a @ Dx): single-pass fp16.
The v = v0 + u@S add rides the PE accumulation chain: an extra
DoubleRow matmul with stationary (128*I, 8*I) and moving (v0h8, v0l8),
where v0h8 = fp8(4*v0), v0l8 = fp8(4*v0 - v0h8); contribution =
512*v0 to within ~2^-9, matching the psum scale of 512*(u@S).

shrink on-chip: psum holds 512*v.
  ACT path: p = relu(psum/512 - th), q = relu(-psum/512 - th) [fp16],
            u8 = p - q (DVE, fp8 out).
  DVE path (4 of 16 tiles, for ACT/DVE balance): u = max(v,th)+min(v,-th):
            t1 = (psum max 512th) * 2^-9, t2 = (psum min -512th) * 2^-9,
            u8 = t1 + t2.

Distribution: data-parallel over batch rows, 8 cores, 2048 rows each;
weights replicated; no collectives.  Per-core loop is split into two
batch halves of 1024 columns so elementwise of one half overlaps the
other half's matmuls; u tiles double-buffer via tile-pool tag rings.
"""

import numpy as np
import ml_dtypes
from contextlib import ExitStack

import concourse.bass as bass
import concourse.bacc as bacc
import concourse.tile as tile
import concourse.mybir as mybir
from concourse.bass import ts, ds

P = 128
NCORES = 8
B_FULL, DIN, DD = 16384, 1024, 2048
BSH = B_FULL // NCORES      # 2048 batch rows per core
HB = BSH // 2               # 1024: batch half (elementwise granularity)
MC = 256                    # matmul moving chunk (DR rhs free = 2*256 = 512)
IT = DD // P                # 16 dict tiles
KW = DIN // P               # 8 d_in tiles
ACH = 1024                  # phase-A batch chunk (= one half)
SA = 512.0                  # S scale (fp8 subnormal escape)
DVE_I = (3, 7, 11, 15)      # prefix tiles shrunk on the DVE path

F8np = ml_dtypes.float8_e4m3
F8 = mybir.dt.float8e4
F16 = mybir.dt.float16
F32 = mybir.dt.float32
ADD = mybir.AluOpType.add
SUB = mybir.AluOpType.subtract
MAX = mybir.AluOpType.max
MIN = mybir.AluOpType.min
MULT = mybir.AluOpType.mult
RELU = mybir.ActivationFunctionType.Relu
COPY = mybir.ActivationFunctionType.Copy
DR = mybir.MatmulPerfMode.DoubleRow

_built = {}


def _build(steps: int):
    """Trace + schedule + compile the SPMD kernel for `steps` unroll steps."""
    assert steps >= 4, "schedule assumes >= 4 steps"
    n1 = steps - 3          # one-pass fp8 steps (incl. step 1 done in A)

    nc = bacc.Bacc("TRN2", target_bir_lowering=False, debug=False, num_devices=NCORES)

    def inp(name, shape, dt):
        return nc.dram_tensor(name, shape, dt, kind="ExternalInput").ap()

    yT16_d = inp("yT16", (DIN, BSH), F16)
    W16_d = inp("W16", (DIN, DD), F16)
    S8h_d = inp("S8h", (DD, DD), F8)
    S8l_d = inp("S8l", (DD, DD), F8)
    Dx8h_d = inp("Dx8h", (DD, DIN), F8)
    Dx8l_d = inp("Dx8l", (DD, DIN), F8)
    eye_d = inp("eye2", (P, 2, P), F8)       # (128*I, 8*I)
    nth_d = inp("nth", (DD,), F32)           # -th
    th512_d = inp("th512", (DD,), F32)       # +512*th
    nth512_d = inp("nth512", (DD,), F32)     # -512*th
    out_d = nc.dram_tensor("out", (BSH, DIN), F32, kind="ExternalOutput").ap()

    with tile.TileContext(nc) as tc, ExitStack() as top:
        dram = top.enter_context(tc.tile_pool(name="dram", bufs=1, space="DRAM"))
        v08_spill = dram.tile([IT, P, 2, BSH], F8)
        a8h_spill = dram.tile([IT, P, BSH], F8)
        a8l_spill = dram.tile([IT, P, BSH], F8)

        thp = top.enter_context(tc.tile_pool(name="thp", bufs=1))
        nth_t = thp.tile([P, IT], F32)
        th512_t = thp.tile([P, IT], F32)
        nth512_t = thp.tile([P, IT], F32)
        eye_t = thp.tile([P, 2, P], F8)
        nc.sync.dma_start(nth_t[:], nth_d.rearrange("(io p) -> p io", p=P))
        nc.sync.dma_start(th512_t[:], th512_d.rearrange("(io p) -> p io", p=P))
        nc.sync.dma_start(nth512_t[:], nth512_d.rearrange("(io p) -> p io", p=P))
        nc.sync.dma_start(eye_t[:], eye_d)

        with ExitStack() as bigctx:
            spool = bigctx.enter_context(tc.tile_pool(name="spool", bufs=1))
            S8h_t = spool.tile([P, IT, DD], F8, name="S8h_t")

            upool = bigctx.enter_context(tc.tile_pool(name="upool", bufs=2))
            psAB = bigctx.enter_context(tc.tile_pool(name="psAB", bufs=4, space="PSUM"))
            u8_h = [upool.tile([P, IT, HB], F8, tag=f"u8h{h}", name=f"u8_h{h}")
                    for h in range(2)]

            v08p = bigctx.enter_context(tc.tile_pool(name="v08p", bufs=3))
            pq = bigctx.enter_context(tc.tile_pool(name="pq", bufs=3))
            uh8_h = [None, None]
            ul8_h = [None, None]
            cur = {}
            # tailp/s8lp opened lazily (first 3-pass step) to keep phase-A SBUF low
            lazy = {}

            def step_begin(k, h):
                one_pass = k <= n1
                split_prod = (k + 1 > n1) and (k < steps)
                final = k == steps
                cur[h] = (u8_h[h], uh8_h[h], ul8_h[h])
                if split_prod:
                    uh8_h[h] = upool.tile([P, IT, HB], F8, tag=f"u8h{h}", name="uh8_n")
                    ul8_h[h] = lazy["tailp"].tile([P, IT, HB], F8, tag=f"ul8h{h}",
                                                  name="ul8_n")
                elif not final:
                    u8_h[h] = upool.tile([P, IT, HB], F8, tag=f"u8h{h}", name="u8_n")

            def step_tile(k, h, i):
                one_pass = k <= n1
                split_prod = (k + 1 > n1) and (k < steps)
                final = k == steps
                hs = ds(h * HB, HB)
                u8_cur, uh8_cur, ul8_cur = cur[h]
                v08c = v08p.tile([P, 2, HB], F8, tag="v08", name="v08c")
                dq = nc.sync if i % 2 else nc.scalar
                dq.dma_start(v08c[:], v08_spill[i, :, :, hs])
                if not one_pass:
                    s8li = lazy["s8lp"].tile([P, IT, P], F8, tag="s8l", name="s8li")
                    dq2 = nc.scalar if i % 2 else nc.sync
                    dq2.dma_start(s8li[:], S8l_d[:, ts(i, P)].rearrange(
                        "(jo p) c -> p jo c", p=P))
                ps = psAB.tile([P, HB], F32, tag="ps", name="psB_t")
                for c in range(HB // MC):
                    cl = ds(c * MC, MC)
                    nc.tensor.matmul(ps[:, cl], eye_t[:], v08c[:, :, cl],
                                     start=True, stop=False, perf_mode=DR)
                    if one_pass:
                        for jp in range(IT // 2):
                            nc.tensor.matmul(
                                ps[:, cl], S8h_t[:, 2 * jp:2 * jp + 2, ts(i, P)],
                                u8_cur[:, 2 * jp:2 * jp + 2, cl],
                                start=False, stop=(jp == IT // 2 - 1), perf_mode=DR)
                    else:
                        for jp in range(IT // 2):
                            nc.tensor.matmul(
                                ps[:, cl], S8h_t[:, 2 * jp:2 * jp + 2, ts(i, P)],
                                uh8_cur[:, 2 * jp:2 * jp + 2, cl],
                                start=False, stop=False, perf_mode=DR)
                        for jp in range(IT // 2):
                            nc.tensor.matmul(
                                ps[:, cl], s8li[:, 2 * jp:2 * jp + 2, :],
                                uh8_cur[:, 2 * jp:2 * jp + 2, cl],
                                start=False, stop=False, perf_mode=DR)
                        for jp in range(IT // 2):
                            nc.tensor.matmul(
                                ps[:, cl], S8h_t[:, 2 * jp:2 * jp + 2, ts(i, P)],
                                ul8_cur[:, 2 * jp:2 * jp + 2, cl],
                                start=False, stop=(jp == IT // 2 - 1), perf_mode=DR)

                # shrink: psum holds 512*v
                if (not split_prod) and (not final) and i in DVE_I:
                    # DVE path: u = max(v,th) + min(v,-th)
                    t1 = pq.tile([P, HB], F16, tag="p", name="t1")
                    t2 = pq.tile([P, HB], F16, tag="q", name="t2")
                    nc.vector.tensor_scalar(
                        t1[:], ps[:], th512_t[:, i:i + 1], 1.0 / SA, MAX, MULT)
                    nc.vector.tensor_scalar(
                        t2[:], ps[:], nth512_t[:, i:i + 1], 1.0 / SA, MIN, MULT)
                    nc.vector.tensor_tensor(u8_h[h][:, i, :], t1[:], t2[:], ADD)
                else:
                    p_t = pq.tile([P, HB], F16, tag="p", name="p_t")
                    q_t = pq.tile([P, HB], F16, tag="q", name="q_t")
                    nc.scalar.activation(p_t[:], ps[:], RELU,
                                         bias=nth_t[:, i:i + 1], scale=1.0 / SA)
                    nc.scalar.activation(q_t[:], ps[:], RELU,
                                         bias=nth_t[:, i:i + 1], scale=-1.0 / SA)
                    if final:
                        u16 = pq.tile([P, HB], F16, tag="u16", name="u16")
                        nc.vector.tensor_tensor(u16[:], p_t[:], q_t[:], SUB)
                        ah8c = lazy["tailp"].tile([P, HB], F8, tag="ah8", name="ah8c")
                        nc.vector.tensor_copy(ah8c[:], u16[:])
                        al8c = lazy["tailp"].tile([P, HB], F8, tag="al8", name="al8c")
                        nc.vector.tensor_tensor(al8c[:], u16[:], ah8c[:], SUB)
                        nc.sync.dma_start(a8h_spill[i, :, hs], ah8c[:])
                        nc.scalar.dma_start(a8l_spill[i, :, hs], al8c[:])
                    elif split_prod:
                        u16 = pq.tile([P, HB], F16, tag="u16", name="u16")
                        nc.vector.tensor_tensor(u16[:], p_t[:], q_t[:], SUB)
                        nc.vector.tensor_copy(uh8_h[h][:, i, :], u16[:])
                        nc.vector.tensor_tensor(ul8_h[h][:, i, :], u16[:],
                                                uh8_h[h][:, i, :], SUB)
                    else:
                        nc.vector.tensor_tensor(u8_h[h][:, i, :], p_t[:], q_t[:], SUB)

            # ---------------- Phase A: v0 = W^T @ y^T; v0 fp8 pair + u_1 ----------------
            with ExitStack() as actx:
                wpool = actx.enter_context(tc.tile_pool(name="wpool", bufs=1))
                ypool = actx.enter_context(tc.tile_pool(name="ypool", bufs=2))
                vtmp = actx.enter_context(tc.tile_pool(name="vtmp", bufs=3))
                pqA = actx.enter_context(tc.tile_pool(name="pqA", bufs=3))

                W16_t = wpool.tile([P, KW, DD], F16, name="W16_t")
                for ko in range(KW):
                    dq = nc.sync if ko % 2 else nc.scalar
                    dq.dma_start(W16_t[:, ko, :], W16_d[ts(ko, P), :])

                for h in range(BSH // ACH):
                    cs = ds(h * ACH, ACH)
                    yc = ypool.tile([P, KW, ACH], F16, tag="yc", name="yc")
                    for ko in range(KW):
                        dq = nc.sync if ko % 2 else nc.scalar
                        dq.dma_start(yc[:, ko, :], yT16_d[ts(ko, P), cs])
                    if h == 0:
                        # S8h needed from step 2 on; queue it behind the first
                        # y chunk so A's first matmuls aren't starved, but well
                        # before the B phase starts
                        for jo in range(IT):
                            dq = nc.sync if jo % 2 else nc.scalar
                            dq.dma_start(S8h_t[:, jo, :], S8h_d[ts(jo, P), :])
                    else:
                        # overlap step-2/h0 with the second A half
                        step_begin(2, 0)
                    for i in range(IT):
                        ps = psAB.tile([P, ACH], F32, tag="ps", name="psA_t")
                        for s in range(2):
                            sl = ds(s * 512, 512)
                            for ko in range(KW):
                                nc.tensor.matmul(ps[:, sl], W16_t[:, ko, ts(i, P)],
                                                 yc[:, ko, sl],
                                                 start=(ko == 0), stop=(ko == KW - 1))
                        # v0 pair: v0h8 = fp8(4*v0), v0l8 = fp8(4*v0 - v0h8)
                        # (eye carries 128x on both -> contribution 512*v0)
                        v0h8c = vtmp.tile([P, ACH], F8, tag="v0h8", name="v0h8c")
                        if i % 2:
                            nc.scalar.activation(v0h8c[:], ps[:], COPY, scale=4.0)
                        else:
                            nc.vector.tensor_scalar(v0h8c[:], ps[:], 4.0, None, MULT)
                        v0l8c = vtmp.tile([P, ACH], F8, tag="v0l8", name="v0l8c")
                        nc.vector.scalar_tensor_tensor(v0l8c[:], ps[:], 4.0, v0h8c[:],
                                                       MULT, SUB)
                        nc.sync.dma_start(v08_spill[i, :, 0, cs], v0h8c[:])
                        nc.scalar.dma_start(v08_spill[i, :, 1, cs], v0l8c[:])
                        # u_1 = shrink(v0):  psum is UNSCALED here (scale=1)
                        p_t = pqA.tile([P, ACH], F16, tag="pA", name="p_t")
                        q_t = pqA.tile([P, ACH], F16, tag="qA", name="q_t")
                        nc.scalar.activation(p_t[:], ps[:], RELU, bias=nth_t[:, i:i + 1],
                                             scale=1.0)
                        nc.scalar.activation(q_t[:], ps[:], RELU, bias=nth_t[:, i:i + 1],
                                             scale=-1.0)
                        nc.vector.tensor_tensor(u8_h[h][:, i, :], p_t[:], q_t[:], SUB)
                        if h == 1:
                            step_tile(2, 0, i)

            # ---------------- Phase B ----------------
            with ExitStack() as bctx:
                lazy["tailp"] = bctx.enter_context(tc.tile_pool(name="tailp", bufs=2))
                lazy["s8lp"] = bctx.enter_context(tc.tile_pool(name="s8lp", bufs=3))

                # step k consumes u_k, produces u_{k+1} (or a16 at k=steps)
                for k in range(2, steps + 1):
                    for h in range(2):
                        if k == 2 and h == 0:
                            continue    # emitted inside phase A
                        step_begin(k, h)
                        for i in range(IT):
                            step_tile(k, h, i)

        # ---------------- Phase C: out = a @ Dx (fp16) ----------------
        with ExitStack() as cctx:
            cpool = cctx.enter_context(tc.tile_pool(name="cpool", bufs=1))
            psC = cctx.enter_context(tc.tile_pool(name="psC", bufs=4, space="PSUM"))
            stC = cctx.enter_context(tc.tile_pool(name="stC", bufs=3))

            CN2 = 256
            Dx8h_t = cpool.tile([P, IT, DIN], F8, name="Dx8h_t")
            Dx8l_t = cpool.tile([P, IT, DIN], F8, name="Dx8l_t")
            for io in range(IT):
                nc.scalar.dma_start(Dx8h_t[:, io, :], Dx8h_d[ts(io, P), :])
                nc.scalar.dma_start(Dx8l_t[:, io, :], Dx8l_d[ts(io, P), :])
            a16p2 = cctx.enter_context(tc.tile_pool(name="a16p2", bufs=3))
            for bt in range(BSH // P):
                ah8bt = a16p2.tile([P, IT, P], F8, tag="ah8bt", name="ah8bt")
                nc.sync.dma_start(ah8bt[:], a8h_spill[:, :, ts(bt, P)].rearrange(
                    "io p b -> p io b"))
                al8bt = a16p2.tile([P, IT, P], F8, tag="al8bt", name="al8bt")
                nc.sync.dma_start(al8bt[:], a8l_spill[:, :, ts(bt, P)].rearrange(
                    "io p b -> p io b"))
                for dn in range(DIN // CN2):
                    dsl = ds(dn * CN2, CN2)
                    ps = psC.tile([P, CN2], F32, tag="psC", name="psC_t")
                    for jp in range(IT // 2):
                        nc.tensor.matmul(ps[:], ah8bt[:, 2 * jp:2 * jp + 2, :],
                                         Dx8h_t[:, 2 * jp:2 * jp + 2, dsl],
                                         start=(jp == 0), stop=False, perf_mode=DR)
                    for jp in range(IT // 2):
                        nc.tensor.matmul(ps[:], ah8bt[:, 2 * jp:2 * jp + 2, :],
                                         Dx8l_t[:, 2 * jp:2 * jp + 2, dsl],
                                         start=False, stop=False, perf_mode=DR)
                    for jp in range(IT // 2):
                        nc.tensor.matmul(ps[:], al8bt[:, 2 * jp:2 * jp + 2, :],
                                         Dx8h_t[:, 2 * jp:2 * jp + 2, dsl],
                                         start=False, stop=(jp == IT // 2 - 1),
                                         perf_mode=DR)
                    st = stC.tile([P, CN2], F32, tag="stC", name="st")
                    nc.scalar.activation(st[:], ps[:], COPY, scale=1.0 / SA)
                    nc.sync.dma_start(out_d[ts(bt, P), dsl], st[:])

    nc.compile()
    return nc


def _prep_in_maps(y, W, Theta, S, Dx):
    y = np.ascontiguousarray(np.asarray(y, dtype=np.float32))
    W = np.asarray(W, dtype=np.float32)
    Theta = np.asarray(Theta, dtype=np.float32)
    S = np.asarray(S, dtype=np.float32)
    Dx = np.asarray(Dx, dtype=np.float32)
    assert y.shape == (B_FULL, DIN) and W.shape == (DIN, DD)
    assert S.shape == (DD, DD) and Dx.shape == (DD, DIN)

    W16 = W.astype(np.float16)
    S512 = S * np.float32(SA)
    S8h = S512.astype(F8np)
    S8l = (S512 - S8h.astype(np.float32)).astype(F8np)
    Dx512 = Dx * np.float32(SA)
    Dx8h = Dx512.astype(F8np)
    Dx8l = (Dx512 - Dx8h.astype(np.float32)).astype(F8np)
    th = (np.maximum(Theta, 0.0) + np.float32(1e-7)).astype(np.float32)
    eye = np.zeros((P, 2, P), np.float32)
    eye[:, 0, :] = np.eye(P) * 128.0
    eye[:, 1, :] = np.eye(P) * 128.0
    eye2 = eye.astype(F8np)
    yT16 = np.ascontiguousarray(y.T).astype(np.float16)

    shared = dict(W16=W16, S8h=S8h, S8l=S8l, Dx8h=Dx8h, Dx8l=Dx8l, eye2=eye2,
                  nth=-th, th512=np.float32(SA) * th, nth512=-np.float32(SA) * th)
    in_maps = []
    for c in range(NCORES):
        sl = slice(c * BSH, (c + 1) * BSH)
        in_maps.append(dict(shared, yT16=np.ascontiguousarray(yT16[:, sl])))
    return in_maps


_sharded_cache = {}


def _get_sharded(steps: int):
    """Build (once) the jitted shard_map executable for the compiled NEFF."""
    if steps in _sharded_cache:
        return _sharded_cache[steps]
    import jax
    from jax.experimental.shard_map import shard_map
    from jax.sharding import Mesh, PartitionSpec
    from concourse import bass2jax

    if steps not in _built:
        _built[steps] = _build(steps)
    nc = _built[steps]
    bass2jax.install_neuronx_cc_hook()
    assert nc.dbg_addr is None
    partition_name = nc.partition_id_tensor.name if nc.partition_id_tensor else None

    in_names, out_names, out_avals, zero_shapes = [], [], [], []
    for alloc in nc.m.functions[0].allocations:
        if not isinstance(alloc, mybir.MemoryLocationSet):
            continue
        name = alloc.memorylocations[0].name
        if alloc.kind == "ExternalInput":
            if name != partition_name:
                in_names.append(name)
        elif alloc.kind == "ExternalOutput":
            out_names.append(name)
            shape = tuple(alloc.tensor_shape)
            dtype = mybir.dt.np(alloc.dtype)
            out_avals.append(jax.core.ShapedArray(shape, dtype))
            zero_shapes.append((shape, dtype))
    n_params = len(in_names)
    n_outs = len(out_names)
    all_in_names = in_names + out_names
    if partition_name is not None:
        all_in_names.append(partition_name)

    def _body(*args):
        operands = list(args)
        if partition_name is not None:
            operands.append(bass2jax.partition_id_tensor())
        outs = bass2jax._bass_exec_p.bind(
            *operands,
            out_avals=tuple(out_avals),
            in_names=tuple(all_in_names),
            out_names=tuple(out_names),
            lowering_input_output_aliases=(),
            sim_require_finite=True,
            sim_require_nnan=True,
            nc=nc,
        )
        return tuple(outs)

    devices = jax.devices()[:NCORES]
    mesh = Mesh(np.asarray(devices), ("core",))
    donate = tuple(range(n_params, n_params + n_outs))
    sharded = jax.jit(
        shard_map(_body, mesh=mesh,
                  in_specs=(PartitionSpec("core"),) * (n_params + n_outs),
                  out_specs=(PartitionSpec("core"),) * n_outs,
                  check_rep=False),
        donate_argnums=donate, keep_unused=True)
    entry = dict(sharded=sharded, in_names=in_names, out_names=out_names,
                 zero_shapes=zero_shapes, mesh=mesh, n_params=n_params)
    _sharded_cache[steps] = entry
    return entry


def _concat_inputs(entry, in_maps):
    return [np.concatenate([np.asarray(in_maps[c][n]) for c in range(NCORES)], axis=0)
            for n in entry["in_names"]]


def _run(entry, concat_in):
    zeros = [np.zeros((NCORES * s[0], *s[1:]), d) for s, d in entry["zero_shapes"]]
    out_arrs = entry["sharded"](*concat_in, *zeros)
    return out_arrs


def kernel(y, W, Theta, S, Dx, unroll_steps):
    steps = int(unroll_steps)
    entry = _get_sharded(steps)
    in_maps = _prep_in_maps(y, W, Theta, S, Dx)
    out_arrs = _run(entry, _concat_inputs(entry, in_maps))
    idx = entry["out_names"].index("out")
    return np.ascontiguousarray(np.asarray(out_arrs[idx]))  # [NCORES*BSH, DIN]


def time_kernel(np_inputs, iters=6):
    """Steady-state wall time per NEFF execution (ns), device-resident inputs."""
    import jax
    from jax.sharding import NamedSharding, PartitionSpec
    steps = int(np_inputs["unroll_steps"])
    entry = _get_sharded(steps)
    in_maps = _prep_in_maps(np_inputs["y"], np_inputs["W"], np_inputs["Theta"],
                            np_inputs["S"], np_inputs["Dx"])
    concat_in = _concat_inputs(entry, in_maps)
    sh = NamedSharding(entry["mesh"], PartitionSpec("core"))
    dev_in = [jax.device_put(a, sh) for a in concat_in]
    import time as _time
    times = []
    for it in range(iters):
        zeros = [jax.device_put(np.zeros((NCORES * s[0], *s[1:]), d), sh)
                 for s, d in entry["zero_shapes"]]
        for z in zeros:
            z.block_until_ready()
        t0 = _time.perf_counter()
        outs = entry["sharded"](*dev_in, *zeros)
        for o in outs:
            o.block_until_ready()
        times.append(_time.perf_counter() - t0)
    best = min(times[1:]) if len(times) > 1 else times[0]
    print("  per-iter times (ms):", [f"{t*1e3:.1f}" for t in times])
    return best * 1e9


if __name__ == "__main__":
    rng = np.random.default_rng(0)
    inputs = dict(
        y=rng.standard_normal((B_FULL, DIN), dtype=np.float32),
        W=(rng.standard_normal((DIN, DD)) * 0.02).astype(np.float32),
        Theta=rng.random(DD, dtype=np.float32),
        S=(rng.standard_normal((DD, DD)) * 0.02).astype(np.float32),
        Dx=(rng.standard_normal((DD, DIN)) * 0.02).astype(np.float32),
        unroll_steps=16,
    )
    out = kernel(**inputs)
    print("out", out.shape, out.dtype, np.abs(out).max())
